# revision 11
# baseline (speedup 1.0000x reference)
"""Causal self-attention (B=1, T=4096, D=1024, H=16, dh=64) on 8 trn2 NeuronCores.

Sharding: tensor-parallel over heads — each core owns 2 of the 16 heads.
Per core: QKV projection (transposed activation layout), RoPE, causal
flash-style attention with transposed score tiles St[k,q] (so the AV matmul
needs no P transposes), softmax denominator via an appended ones-column in V,
out-projection against this core's W_out column slice -> partial output
[T, D] in bf16.  Host sums the 8 partials.

v2: softmax exp is split across Scalar (LUT exp) + Vector + GpSimd
(Schraudolph bf16 bit-trick exp via tensor_scalar -> int16 bitcast), score
PSUM double-buffered, all matmul moving operands bf16 (full PE rate),
softmax recip via reciprocal_approx_fast (no DRAM bounce), bf16 output.
"""

import sys

sys.path.insert(0, "/opt/trn_rl_repo")

import numpy as np

import concourse.bass as bass
import concourse.tile as tile
from concourse import bacc, mybir
from concourse.alu_op_type import AluOpType
from concourse.bass_utils import run_bass_kernel_spmd

T = 4096
D = 1024
H = 16
DH = 64
NC = 8
HL = H // NC  # heads per core (2)
DL = HL * DH  # local feature width (128)

F32 = mybir.dt.float32
BF16 = mybir.dt.bfloat16
I16 = mybir.dt.int16

# exp(s/8) = 2^(s * 0.125 * log2 e); bf16 bits = round(t*128 + 16256 - C)
EXP_SCALE = 0.125 * 1.4426950408889634 * 128.0  # 23.0831
EXP_BIAS = 16256.0 - 7.4  # Schraudolph-centred


def build_nc():
    nc = bacc.Bacc(
        "TRN2", target_bir_lowering=False, debug=False, num_devices=NC
    )

    # ---- DRAM I/O -------------------------------------------------------
    xT_d = nc.dram_tensor("xT", [D, T], BF16, kind="ExternalInput").ap()
    wqkvT_d = nc.dram_tensor("wqkvT", [D, 3 * DL], BF16, kind="ExternalInput").ap()
    woutT_d = nc.dram_tensor("woutT", [DL, D], BF16, kind="ExternalInput").ap()
    cos2_d = nc.dram_tensor("cos2", [DL, T], BF16, kind="ExternalInput").ap()
    sin2_d = nc.dram_tensor("sin2", [DL, T], BF16, kind="ExternalInput").ap()
    p128_d = nc.dram_tensor("p128", [DL, DL], BF16, kind="ExternalInput").ap()
    ident_d = nc.dram_tensor("ident", [128, 128], F32, kind="ExternalInput").ap()
    e2_d = nc.dram_tensor("e2", [HL, 128], F32, kind="ExternalInput").ap()
    # 4 diagonal-block masks [128, 512]: mask_j[k, q] = 1 iff q >= j*128 + k
    dmask_d = nc.dram_tensor("dmask", [128, 4 * 512], BF16, kind="ExternalInput").ap()
    out_d = nc.dram_tensor("outp", [T, D], BF16, kind="ExternalOutput").ap()

    NCH = 4  # T-chunks of 1024 for the QKV projection
    CW = 1024
    VBLK = 130  # v-nat block layout: [v_h0(64) | 1 | v_h1(64) | 1]

    with tile.TileContext(nc) as tc:
        with tc.tile_pool(name="consts", bufs=1) as cpool, \
             tc.tile_pool(name="persist", bufs=1) as ppool:
            # ---- constants needed immediately (QKV weights), spread over
            # engine DMA queues so the first matmul can start early --------
            wt = []
            dma_engines = [nc.sync, nc.scalar, nc.gpsimd]
            for d in range(8):
                w = cpool.tile([128, 3 * DL], BF16, tag=f"wt{d}")
                dma_engines[d % 3].dma_start(
                    out=w[:], in_=wqkvT_d[d * 128:(d + 1) * 128, :])
                wt.append(w)
            p128 = cpool.tile([DL, DL], BF16, tag="p128")
            nc.gpsimd.dma_start(out=p128[:], in_=p128_d)
            ident = cpool.tile([128, 128], F32, tag="ident")
            nc.gpsimd.dma_start(out=ident[:], in_=ident_d)

            # ---- persistent activations --------------------------------
            qT = ppool.tile([DL, T], BF16, tag="qT")
            kT = ppool.tile([DL, T], BF16, tag="kT")
            vnat = ppool.tile([128, (T // 128) * VBLK], BF16, tag="vnat")
            attnT = ppool.tile([DL, T], BF16, tag="attnT")

            # ones columns of the v-nat layout (cols 64 and 129 of each block)
            ones_sb = cpool.tile([128, T // 128], F32, tag="ones_sb")
            nc.gpsimd.memset(ones_sb[:], 1.0)
            vone = vnat[:].rearrange("p (b c) -> p b c", c=VBLK)
            nc.vector.tensor_copy(vone[:, :, 64:65], ones_sb[:].rearrange("p (b c) -> p b c", c=1))
            nc.vector.tensor_copy(vone[:, :, 129:130], ones_sb[:].rearrange("p (b c) -> p b c", c=1))

            # ---- remaining constants (deferred so x/W DMAs go first) ---
            cos2 = cpool.tile([DL, T], BF16, tag="cos2")
            sin2 = cpool.tile([DL, T], BF16, tag="sin2")
            woutT = cpool.tile([DL, D], BF16, tag="woutT")
            e2 = cpool.tile([HL, 128], F32, tag="e2")
            dmask = cpool.tile([128, 4 * 512], BF16, tag="dmask")

            # ================= Phase A: QKV + RoPE ======================
            with tc.tile_pool(name="xp", bufs=2) as xpool, \
                 tc.tile_pool(name="tmpa", bufs=3) as tpool, \
                 tc.tile_pool(name="psA", bufs=2, space="PSUM") as psA:
                for c in range(NCH):
                    s = c * CW
                    xt = xpool.tile([128, 8 * CW], BF16, tag="xchunk")
                    for d in range(8):
                        eng = nc.sync if d % 2 == 0 else nc.scalar
                        eng.dma_start(
                            out=xt[:, d * CW:(d + 1) * CW],
                            in_=xT_d[d * 128:(d + 1) * 128, s:s + CW],
                        )
                    if c == 0:
                        # deferred constant loads, off the x-chunk queues
                        nc.gpsimd.dma_start(out=cos2[:], in_=cos2_d)
                        nc.gpsimd.dma_start(out=sin2[:], in_=sin2_d)
                        nc.gpsimd.dma_start(out=dmask[:], in_=dmask_d)
                        nc.gpsimd.dma_start(out=e2[:], in_=e2_d)
                        nc.gpsimd.dma_start(out=woutT[:], in_=woutT_d)

                    def xs(d):
                        return xt[:, d * CW:(d + 1) * CW]

                    # qT / kT with RoPE
                    for idx, dst in ((0, qT), (1, kT)):
                        pp = psA.tile([128, CW], F32, tag="qkvps")
                        for d in range(8):
                            for hf in range(2):
                                fsl = slice(hf * 512, (hf + 1) * 512)
                                nc.tensor.matmul(
                                    pp[:, fsl],
                                    lhsT=wt[d][:, idx * DL:(idx + 1) * DL],
                                    rhs=xs(d)[:, fsl],
                                    start=(d == 0),
                                    stop=(d == 7),
                                )
                        praw = tpool.tile([128, CW], BF16, tag="praw")
                        nc.vector.tensor_copy(praw[:], pp[:])
                        dstv = dst[:, s:s + CW]
                        nc.vector.tensor_mul(dstv, praw[:], cos2[:, s:s + CW])
                        rtmp = tpool.tile([128, CW], BF16, tag="rtmp")
                        for half in range(2):
                            hsl = slice(half * 512, (half + 1) * 512)
                            rot = psA.tile([128, 512], F32, tag="rotps")
                            nc.tensor.matmul(
                                rot[:], lhsT=p128[:], rhs=praw[:, hsl],
                                start=True, stop=True,
                            )
                            nc.vector.tensor_mul(
                                rtmp[:, hsl], rot[:],
                                sin2[:, s + half * 512:s + (half + 1) * 512])
                        nc.gpsimd.tensor_add(dstv, dstv, rtmp[:])

                    # v: compute vT then PE-transpose to natural layout
                    vp = psA.tile([128, CW], F32, tag="qkvps")
                    for d in range(8):
                        for hf in range(2):
                            fsl = slice(hf * 512, (hf + 1) * 512)
                            nc.tensor.matmul(
                                vp[:, fsl], lhsT=wt[d][:, 2 * DL:3 * DL],
                                rhs=xs(d)[:, fsl], start=(d == 0), stop=(d == 7),
                            )
                    vtmp = tpool.tile([128, CW], F32, tag="vtmp")
                    nc.scalar.copy(vtmp[:], vp[:])
                    for b in range(CW // 128):
                        kb = (s // 128) + b
                        tp = psA.tile([128, 128], F32, tag="vtps")
                        nc.tensor.transpose(
                            tp[:], vtmp[:, b * 128:(b + 1) * 128], ident[:]
                        )
                        o = kb * VBLK
                        nc.vector.tensor_copy(vnat[:, o:o + 64], tp[:, 0:64])
                        nc.vector.tensor_copy(vnat[:, o + 65:o + 129], tp[:, 64:128])

            # ====== Phase B+C: attention, normalize, out-projection =====
            # per q-chunk: S -> exp (3 engines) -> mask -> AV, per 128-k
            # block; normalize + out-projection of the PREVIOUS chunk runs
            # under this chunk's matmul stream so the PE never stalls on
            # the softmax-recip latency chain.
            # gpsimd cannot touch PSUM, so exp (PSUM-read) splits across
            # scalar (LUT) and vector (bit-trick); gpsimd gets the SBUF-only
            # work: causal masks and the normalize multiplies.
            EXP_PAT = "svsvsvsvs"  # scalar 5/9, vector 4/9
            with tc.tile_pool(name="ptp", bufs=3) as ptpool, \
                 tc.tile_pool(name="evp", bufs=2) as evpool, \
                 tc.tile_pool(name="nrm", bufs=2) as npool, \
                 tc.tile_pool(name="op", bufs=3) as opool, \
                 tc.tile_pool(name="psAT", bufs=1, space="PSUM") as psAT, \
                 tc.tile_pool(name="psST", bufs=2, space="PSUM") as psST, \
                 tc.tile_pool(name="psO", bufs=2, space="PSUM") as psO:
                rf_prev = None
                rf_p = None
                for qc in range(9):
                  if qc < 8:
                    q0 = qc * 512
                    kmax = 4 * (qc + 1)
                    ats = [psAT.tile([DH + 1, 512], F32, tag=f"atps{h}", name=f"at{h}")
                           for h in range(HL)]
                    for kb in range(kmax):
                        sts = [psST.tile([128, 512], F32, tag=f"stps{h}", name=f"st{h}")
                               for h in range(HL)]
                        # S matmuls: interleave heads so the two K=64
                        # matmuls occupy PE row groups 0-63 / 64-127.
                        for h in range(HL):
                            hs = h * DH
                            nc.tensor.matmul(
                                sts[h][:],
                                lhsT=kT[hs:hs + DH, kb * 128:(kb + 1) * 128],
                                rhs=qT[hs:hs + DH, q0:q0 + 512],
                                start=True, stop=True,
                            )
                        pts = []
                        for h in range(HL):
                            pt = ptpool.tile([128, 512], BF16, tag=f"pt{h}")
                            e = EXP_PAT[(2 * kb + h + qc) % 9]
                            if e == "s":
                                nc.scalar.activation(
                                    pt[:], sts[h][:],
                                    mybir.ActivationFunctionType.Exp,
                                    scale=0.125,
                                )
                            else:
                                nc.vector.tensor_scalar(
                                    out=pt[:].bitcast(I16), in0=sts[h][:],
                                    scalar1=EXP_SCALE, scalar2=EXP_BIAS,
                                    op0=AluOpType.mult, op1=AluOpType.add,
                                )
                            j = kb - 4 * qc
                            if j >= 0:
                                nc.gpsimd.tensor_mul(
                                    pt[:], pt[:],
                                    dmask[:, j * 512:(j + 1) * 512],
                                )
                            pts.append(pt)
                        for h in range(HL):
                            o = kb * VBLK + h * 65
                            nc.tensor.matmul(
                                ats[h][:],
                                lhsT=vnat[:, o:o + 65],
                                rhs=pts[h][:],
                                start=(kb == 0), stop=(kb == kmax - 1),
                                skip_group_check=True,
                            )
                    # evacuate attn rows (bf16) + sums; recip on-chip.
                    sums2 = npool.tile([HL, 512], F32, tag="sums2")
                    rf = npool.tile([HL, 512], F32, tag="rf")
                    ev = evpool.tile([DH, 512], BF16, tag="ev1")
                    ss = npool.tile([DH + 1, 2 * 512], F32, tag="ss")
                    nc.vector.tensor_copy(attnT[0:DH, q0:q0 + 512], ats[0][0:DH, :])
                    nc.vector.tensor_copy(ev[:], ats[1][0:DH, :])
                    nc.sync.dma_start(
                        out=attnT[DH:2 * DH, q0:q0 + 512], in_=ev[:])
                    for h in range(HL):
                        nc.vector.tensor_copy(
                            ss[DH:DH + 1, h * 512:(h + 1) * 512],
                            ats[h][DH:DH + 1, :])
                    for h in range(HL):
                        nc.scalar.dma_start(
                            out=sums2[h:h + 1, :],
                            in_=ss[DH:DH + 1, h * 512:(h + 1) * 512],
                        )
                    with nc.allow_low_precision(reason="softmax recip"):
                        nc.vector.reciprocal_approx_fast(out=rf[:], in_=sums2[:])
                    rf_prev = rf
                  if qc > 0:
                    # normalize + project the PREVIOUS chunk (its recip
                    # chain completed under this chunk's matmuls).
                    qp0 = (qc - 1) * 512
                    rb = psO.tile([128, 512], F32, tag="ops")
                    nc.tensor.matmul(
                        rb[:], lhsT=e2[:], rhs=rf_p[:],
                        start=True, stop=True,
                    )
                    rbs = npool.tile([128, 512], BF16, tag="rbs")
                    nc.vector.tensor_copy(rbs[:], rb[:])
                    nc.gpsimd.tensor_mul(
                        attnT[:, qp0:qp0 + 512], attnT[:, qp0:qp0 + 512], rbs[:],
                    )
                    for tbl in range(4):
                        tb = (qc - 1) * 4 + tbl
                        osb = opool.tile([128, D], BF16, tag="osb")
                        for ec in range(2):
                            op = psO.tile([128, 512], F32, tag="ops")
                            nc.tensor.matmul(
                                op[:],
                                lhsT=attnT[:, tb * 128:(tb + 1) * 128],
                                rhs=woutT[:, ec * 512:(ec + 1) * 512],
                                start=True, stop=True,
                            )
                            nc.vector.tensor_copy(
                                osb[:, ec * 512:(ec + 1) * 512], op[:])
                        nc.sync.dma_start(
                            out=out_d[tb * 128:(tb + 1) * 128, :], in_=osb[:]
                        )
                  rf_p = rf_prev

    nc.compile()
    return nc


def _host_constants():
    import ml_dtypes
    inv_freq = 1.0 / (10000.0 ** (np.arange(0, DH, 2, dtype=np.float64) / DH))
    t = np.arange(T, dtype=np.float64)
    freqs = np.outer(t, inv_freq)  # [T, 32]
    emb = np.concatenate([freqs, freqs], axis=-1)  # [T, 64]
    cos = np.cos(emb).astype(np.float32).T  # [64, T]
    sin = np.sin(emb).astype(np.float32).T  # [64, T]
    sinS = sin.copy()
    sinS[0:DH // 2] *= -1.0  # fold rotate_half's negation into the table
    cos2 = np.ascontiguousarray(np.tile(cos, (HL, 1))).astype(ml_dtypes.bfloat16)
    sin2 = np.ascontiguousarray(np.tile(sinS, (HL, 1))).astype(ml_dtypes.bfloat16)

    # swap-halves permutation (per 64-row head block), symmetric
    p1 = np.zeros((DH, DH), np.float32)
    half = DH // 2
    p1[np.arange(half), np.arange(half) + half] = 1.0
    p1[np.arange(half) + half, np.arange(half)] = 1.0
    p128 = np.block([
        [p1, np.zeros((DH, DH), np.float32)],
        [np.zeros((DH, DH), np.float32), p1],
    ]).astype(ml_dtypes.bfloat16)

    ident = np.eye(128, dtype=np.float32)

    e2 = np.zeros((HL, 128), np.float32)
    for h in range(HL):
        e2[h, h * DH:(h + 1) * DH] = 1.0

    # diag masks [128, 4*512]: mask_j[k, q] = 1 iff q >= j*128 + k
    dmask = np.zeros((128, 4, 512), np.float32)
    kk = np.arange(128)[:, None]
    qq = np.arange(512)[None, :]
    for j in range(4):
        dmask[:, j, :] = (qq >= j * 128 + kk).astype(np.float32)
    dmask = np.ascontiguousarray(dmask.reshape(128, 4 * 512)).astype(
        ml_dtypes.bfloat16)
    return cos2, sin2, p128, ident, e2, dmask


_NC_CACHE = None


def _get_nc():
    global _NC_CACHE
    if _NC_CACHE is None:
        _NC_CACHE = build_nc()
    return _NC_CACHE


def _in_maps(x, W_qkv, W_out):
    import ml_dtypes
    x2 = np.asarray(x, np.float32).reshape(T, D)
    W_qkv = np.asarray(W_qkv, np.float32)
    W_out = np.asarray(W_out, np.float32)
    xT = np.ascontiguousarray(x2.T).astype(ml_dtypes.bfloat16)
    cos2, sin2, p128, ident, e2, dmask = _host_constants()

    Wq, Wk, Wv = W_qkv[0:D], W_qkv[D:2 * D], W_qkv[2 * D:3 * D]
    in_maps = []
    for c in range(NC):
        h0, h1 = HL * c, HL * c + 1
        rows = []
        for Wp in (Wq, Wk, Wv):
            rows.append(Wp[h0 * DH:(h0 + 1) * DH])
            rows.append(Wp[h1 * DH:(h1 + 1) * DH])
        wqkvT = np.ascontiguousarray(
            np.concatenate(rows, axis=0).T).astype(ml_dtypes.bfloat16)
        cols = np.r_[h0 * DH:(h0 + 1) * DH, h1 * DH:(h1 + 1) * DH]
        woutT = np.ascontiguousarray(W_out[:, cols].T).astype(
            ml_dtypes.bfloat16)  # [128, D]
        in_maps.append({
            "xT": xT, "wqkvT": wqkvT, "woutT": woutT,
            "cos2": cos2, "sin2": sin2, "p128": p128,
            "ident": ident, "e2": e2, "dmask": dmask,
        })
    return in_maps


def _run(x, W_qkv, W_out, **spmd_kwargs):
    nc = _get_nc()
    res = run_bass_kernel_spmd(
        nc, _in_maps(x, W_qkv, W_out), core_ids=list(range(NC)), **spmd_kwargs
    )
    out = res.results[0]["outp"].astype(np.float64)
    for c in range(1, NC):
        out += res.results[c]["outp"].astype(np.float64)
    return out.astype(np.float32).reshape(1, T, D), res


def kernel(x, W_qkv, W_out):
    out, _ = _run(x, W_qkv, W_out)
    return out
